# revision 7
# baseline (speedup 1.0000x reference)
"""Trainium2 Bass kernel for nn_CausalMultimodal (gnn_message_passing).

Math (per batch row b, fully row-local so batch shards freely over 8 cores):
    mask[i,j]  = (matrix*(matrix>0.1))[i,j] > 0.1
    agg[b,d]   = (Z[b,:] @ mask[d,:]) / count[d]   (0 when count==0)
    hidden     = relu(Z[b,d]*W1[d,0,h] + agg[b,d]*W1[d,1,h] + b1[d,h])
    E[b,d]     = sum_h hidden[b,d,h]*W2[d,h] + b2[d]

Since agg = Z @ M2 with M2[j,d] = mask[d,j]/count[d], the whole first layer
folds into one 32x128 matrix A computed host-side from the tiny params:
U[b, 32h+d] = (Z @ A)[b, 32h+d]; then E = W2sel.T @ relu(U + b1) + b2 with
W2sel (128,32) block-sparse.

Device dataflow per core (B/8 rows), per (128, F) megatile:
  contiguous DMA -> SBUF natural layout -> DVE 32x32 block transpose (the
  batch<->dim transpose; block permutation folded into the row assignment)
  -> PE mm1 (4x row-tiled K=32 bf16) -> ACT/DVE relu PSUM->SBUF (+b1 bias)
  -> PE mm3 (4x col-tiled M=32 bf16) -> DVE block transpose PSUM->SBUF ->
  contiguous DMA out.  PREC>=1 splits Z and A into bf16 hi+lo pairs
  (3 accumulating matmuls) so mm1 is fp32-accurate; PREC=2 also splits W2.
"""

import os

import ml_dtypes
import numpy as np

import concourse.bacc as bacc
import concourse.tile as tile
from concourse import mybir
from concourse import bass_utils

B_TOTAL, D, H = 1048576, 32, 4
NCORES = 8
R = B_TOTAL // NCORES  # rows per core
BF16 = ml_dtypes.bfloat16

F = int(os.environ.get("NNK_F", "2048"))  # megatile free size (cols)
ROWS_PER_MT = 4 * F
MM_MODE = os.environ.get("NNK_MM", "bf16")  # bf16 | f32
PREC = int(os.environ.get("NNK_PREC", "1"))  # 0: single bf16; 1: split Z+A; 2: +W2
DVE_RELU = int(os.environ.get("NNK_DVE_RELU", "1"))  # relu strips (of 4) on DVE
PSUM_T = int(os.environ.get("NNK_PSUMT", "1"))  # StreamTranspose direct from PSUM

_module_cache = {}


def _build_module(rows, b2_zero):
    key = (rows, b2_zero, F, MM_MODE, PREC, DVE_RELU, PSUM_T)
    if key in _module_cache:
        return _module_cache[key]

    f32 = mybir.dt.float32
    bf = mybir.dt.bfloat16
    ddt = bf if MM_MODE == "bf16" else f32
    split_z = MM_MODE == "bf16" and PREC >= 1
    split_w = MM_MODE == "bf16" and PREC >= 2

    nc = bacc.Bacc("TRN2", target_bir_lowering=False, debug=False,
                   num_devices=NCORES)

    zs = ["ZH", "ZL"] if split_z else ["ZH"]
    Zin = {n: nc.dram_tensor(n, (rows, D), ddt, kind="ExternalInput").ap()
           for n in zs}
    As = ["A4H", "A4L"] if split_z else ["A4H"]
    Ain = {n: nc.dram_tensor(n, (128, 128), ddt, kind="ExternalInput").ap()
           for n in As}
    Ws = ["W2H", "W2L"] if split_w else ["W2H"]
    Win = {n: nc.dram_tensor(n, (128, D), ddt, kind="ExternalInput").ap()
           for n in Ws}
    B1V = nc.dram_tensor("B1V", (128, 1), f32, kind="ExternalInput").ap()
    B2V = nc.dram_tensor("B2V", (128, 1), f32, kind="ExternalInput").ap()
    E = nc.dram_tensor("E", (rows, D), f32, kind="ExternalOutput").ap()

    nmt = rows // ROWS_PER_MT
    assert nmt * ROWS_PER_MT == rows
    Zv = {n: ap.rearrange("(n p u) j -> n p (u j)", p=128, u=F // D)
          for n, ap in Zin.items()}
    Ev = E.rearrange("(n p u) j -> n p (u j)", p=128, u=F // D)

    nblk = F // 512

    with tile.TileContext(nc) as tc:
        with (
            tc.tile_pool(name="const", bufs=1) as constp,
            tc.tile_pool(name="zn", bufs=2) as znp,
            tc.tile_pool(name="zt", bufs=2) as ztp,
            tc.tile_pool(name="vv", bufs=2) as vp,
            tc.tile_pool(name="en", bufs=2) as enp,
            tc.tile_pool(name="pu", bufs=1, space="PSUM") as pup,
            tc.tile_pool(name="pe", bufs=3, space="PSUM") as pep,
        ):
            acst = {}
            for n, ap in Ain.items():
                t = constp.tile([128, 128], ddt, tag=n, name=f"c{n}")
                nc.sync.dma_start(out=t, in_=ap)
                acst[n] = t
            wcst = {}
            for n, ap in Win.items():
                t = constp.tile([128, D], ddt, tag=n, name=f"c{n}")
                nc.sync.dma_start(out=t, in_=ap)
                wcst[n] = t
            b1v = constp.tile([128, 1], f32)
            nc.sync.dma_start(out=b1v, in_=B1V)
            b2v = constp.tile([128, 1], f32)
            nc.sync.dma_start(out=b2v, in_=B2V)

            for i in range(nmt):
                zts = {}
                for n in zs:
                    znat = znp.tile([128, F], ddt, tag=f"zn{n}")
                    nc.sync.dma_start(out=znat, in_=Zv[n][i])
                    zt = ztp.tile([128, F], ddt, tag=f"zt{n}")
                    nc.vector.transpose(zt, znat)
                    zts[n] = zt
                # mm1 accumulation passes: AH@ZH (+ AH@ZL + AL@ZH)
                passes1 = [("A4H", "ZH")]
                if split_z:
                    passes1 += [("A4H", "ZL"), ("A4L", "ZH")]
                passes3 = ["W2H"] + (["W2L"] if split_w else [])

                enat = enp.tile([128, F], f32)
                for t in range(nblk):
                    sl = slice(512 * t, 512 * (t + 1))
                    ugs = [pup.tile([128, 512], f32, tag=f"ug{a}",
                                    name=f"ug{a}") for a in range(4)]
                    for pi, (wn, zn_) in enumerate(passes1):
                        w = acst[wn]
                        zt_ = zts[zn_]
                        for a in range(4):
                            nc.tensor.matmul(
                                ugs[a],
                                lhsT=w[32 * a:32 * (a + 1), :],
                                rhs=zt_[32 * a:32 * (a + 1), sl],
                                start=(pi == 0),
                                stop=(pi == len(passes1) - 1),
                                tile_position=(32 * a, 0),
                            )
                    vgs = []
                    for a in range(4):
                        vg = vp.tile([128, 512], ddt, tag=f"vg{a}")
                        if a >= 4 - DVE_RELU:
                            nc.vector.tensor_scalar(
                                vg, ugs[a], b1v, 0.0,
                                mybir.AluOpType.add, mybir.AluOpType.max)
                        else:
                            nc.scalar.activation(
                                vg, ugs[a], mybir.ActivationFunctionType.Relu,
                                bias=b1v, scale=1.0)
                        vgs.append(vg)
                    eps = pep.tile([128, 512], f32)
                    for a in range(4):
                        for pi, wn in enumerate(passes3):
                            nc.tensor.matmul(
                                eps[32 * a:32 * (a + 1), :],
                                lhsT=wcst[wn],
                                rhs=vgs[a],
                                start=(pi == 0),
                                stop=(pi == len(passes3) - 1),
                                tile_position=(0, 32 * a),
                            )
                    if b2_zero and PSUM_T:
                        nc.vector.transpose(enat[:, sl], eps)
                    else:
                        ecp = vp.tile([128, 512], f32, tag="ecp")
                        nc.scalar.activation(
                            ecp, eps, mybir.ActivationFunctionType.Identity,
                            bias=b2v, scale=1.0)
                        nc.vector.transpose(enat[:, sl], ecp)
                nc.sync.dma_start(out=Ev[i], in_=enat)

    nc.compile()
    _module_cache[key] = nc
    return nc


def _fold_params(matrix, W1, b1, W2, b2):
    """Host-side fold of the tiny params into A4/W2S/B1V/B2V (a few KB)."""
    matrix = np.asarray(matrix, np.float32)
    W1 = np.asarray(W1, np.float32)
    b1 = np.asarray(b1, np.float32)
    W2 = np.asarray(W2, np.float32)
    b2 = np.asarray(b2, np.float32)

    alpha_est = matrix * (matrix > np.float32(0.1)).astype(np.float32)
    mask = (alpha_est > np.float32(0.1)).astype(np.float32)  # (D, D)
    cnt = mask.sum(axis=1)  # (D,)
    scale = np.where(cnt > 0, np.float32(1.0) / np.maximum(cnt, 1.0),
                     np.float32(0.0)).astype(np.float32)
    M2 = (mask.T * scale[None, :]).astype(np.float32)  # M2[j,d]

    A = np.zeros((D, D * H), np.float32)
    for h in range(H):
        Ah = M2 * W1[None, :, 1, h]  # (j, d): M2[j,d] * W1[d,1,h]
        Ah[np.arange(D), np.arange(D)] += W1[:, 0, h]
        A[:, D * h:D * (h + 1)] = Ah
    A4 = np.ascontiguousarray(np.tile(A, (4, 1)))  # (128, 128)

    W2S = np.zeros((D * H, D), np.float32)
    W2S[np.arange(D * H), np.tile(np.arange(D), H)] = W2.T.reshape(-1)
    B1V = np.ascontiguousarray(b1.T.reshape(D * H, 1))
    B2V = np.ascontiguousarray(np.tile(b2, H).reshape(D * H, 1))
    b2_zero = not np.any(b2)
    return A4, W2S, B1V, B2V, b2_zero


def _split_bf16(x):
    hi = x.astype(BF16)
    lo = (x - hi.astype(np.float32)).astype(BF16)
    return np.ascontiguousarray(hi), np.ascontiguousarray(lo)


def _run(Z, matrix, W1, b1, W2, b2, trace=False):
    Z = np.ascontiguousarray(np.asarray(Z, np.float32))
    assert Z.shape == (B_TOTAL, D), Z.shape
    A4, W2S, B1V, B2V, b2_zero = _fold_params(matrix, W1, b1, W2, b2)
    nc = _build_module(R, b2_zero)

    cst = {"B1V": B1V, "B2V": B2V}
    if MM_MODE == "bf16":
        if PREC >= 1:
            zh, zl = _split_bf16(Z)
            zdata = {"ZH": zh, "ZL": zl}
            cst["A4H"], cst["A4L"] = _split_bf16(A4)
        else:
            zdata = {"ZH": np.ascontiguousarray(Z.astype(BF16))}
            cst["A4H"] = np.ascontiguousarray(A4.astype(BF16))
        if PREC >= 2:
            cst["W2H"], cst["W2L"] = _split_bf16(W2S)
        else:
            cst["W2H"] = np.ascontiguousarray(W2S.astype(BF16))
    else:
        zdata = {"ZH": Z}
        cst["A4H"] = A4
        cst["W2H"] = W2S

    in_maps = [
        {**cst, **{n: z[c * R:(c + 1) * R] for n, z in zdata.items()}}
        for c in range(NCORES)
    ]
    res = bass_utils.run_bass_kernel_spmd(
        nc, in_maps, core_ids=list(range(NCORES)), trace=trace)
    out = np.concatenate([r["E"] for r in res.results], axis=0)
    return out, res


def kernel(Z, matrix, W1, b1, W2, b2):
    out, _ = _run(Z, matrix, W1, b1, W2, b2, trace=False)
    return out


# revision 10
# speedup vs baseline: 1.6723x; 1.6723x over previous
"""Trainium2 Bass kernel for nn_CausalMultimodal (gnn_message_passing).

Math (per batch row b, fully row-local so batch shards freely over 8 cores):
    mask[i,j]  = (matrix*(matrix>0.1))[i,j] > 0.1
    agg[b,d]   = (Z[b,:] @ mask[d,:]) / count[d]   (0 when count==0)
    hidden     = relu(Z[b,d]*W1[d,0,h] + agg[b,d]*W1[d,1,h] + b1[d,h])
    E[b,d]     = sum_h hidden[b,d,h]*W2[d,h] + b2[d]

Since agg = Z @ M2 with M2[j,d] = mask[d,j]/count[d], the whole first layer
folds into one 32x128 matrix A computed host-side from the tiny params:
U[b, 32h+d] = (Z @ A)[b, 32h+d]; then E = W2sel.T @ relu(U + b1) + b2 with
W2sel (128,32) block-sparse.

Device dataflow per core (B/8 rows), per (128, F) megatile:
  contiguous DMA -> SBUF natural layout -> DVE 32x32 block transpose (the
  batch<->dim transpose; block permutation folded into the row assignment)
  -> PE mm1 (4x row-tiled K=32 bf16) -> ACT/DVE relu PSUM->SBUF (+b1 bias)
  -> PE mm3 (4x col-tiled M=32 bf16) -> DVE block transpose PSUM->SBUF ->
  contiguous DMA out.  PREC>=1 splits Z and A into bf16 hi+lo pairs
  (3 accumulating matmuls) so mm1 is fp32-accurate; PREC=2 also splits W2.
"""

import os

import ml_dtypes
import numpy as np

import concourse.bacc as bacc
import concourse.tile as tile
from concourse import mybir
from concourse import bass_utils

B_TOTAL, D, H = 1048576, 32, 4
NCORES = 8
R = B_TOTAL // NCORES  # rows per core
BF16 = ml_dtypes.bfloat16

F = int(os.environ.get("NNK_F", "2048"))  # megatile free size (cols)
ROWS_PER_MT = 4 * F
MM_MODE = os.environ.get("NNK_MM", "bf16")  # bf16 | f32
PREC = int(os.environ.get("NNK_PREC", "1"))  # 0 none; 1 A-split; 2 Z+A; 3 Z+A+W2
DVE_RELU = int(os.environ.get("NNK_DVE_RELU", "1"))  # relu strips (of 4) on DVE
PSUM_T = int(os.environ.get("NNK_PSUMT", "1"))  # StreamTranspose direct from PSUM

_module_cache = {}


def _build_module(rows, b2_zero):
    key = (rows, b2_zero, F, MM_MODE, PREC, DVE_RELU, PSUM_T)
    if key in _module_cache:
        return _module_cache[key]

    f32 = mybir.dt.float32
    bf = mybir.dt.bfloat16
    ddt = bf if MM_MODE == "bf16" else f32
    split_a = MM_MODE == "bf16" and PREC >= 1
    split_z = MM_MODE == "bf16" and PREC >= 2
    split_w = MM_MODE == "bf16" and PREC >= 3

    nc = bacc.Bacc("TRN2", target_bir_lowering=False, debug=False,
                   num_devices=NCORES)

    zs = ["ZH", "ZL"] if split_z else ["ZH"]
    Zin = {n: nc.dram_tensor(n, (rows, D), ddt, kind="ExternalInput").ap()
           for n in zs}
    As = ["A4H", "A4L"] if split_a else ["A4H"]
    Ain = {n: nc.dram_tensor(n, (128, 128), ddt, kind="ExternalInput").ap()
           for n in As}
    Ws = ["W2H", "W2L"] if split_w else ["W2H"]
    Win = {n: nc.dram_tensor(n, (128, D), ddt, kind="ExternalInput").ap()
           for n in Ws}
    B1V = nc.dram_tensor("B1V", (128, 1), f32, kind="ExternalInput").ap()
    B2V = nc.dram_tensor("B2V", (128, 1), f32, kind="ExternalInput").ap()
    E = nc.dram_tensor("E", (rows, D), f32, kind="ExternalOutput").ap()

    nmt = rows // ROWS_PER_MT
    assert nmt * ROWS_PER_MT == rows
    Zv = {n: ap.rearrange("(n p u) j -> n p (u j)", p=128, u=F // D)
          for n, ap in Zin.items()}
    Ev = E.rearrange("(n p u) j -> n p (u j)", p=128, u=F // D)

    nblk = F // 512

    with tile.TileContext(nc) as tc:
        with (
            tc.tile_pool(name="const", bufs=1) as constp,
            tc.tile_pool(name="zn", bufs=2) as znp,
            tc.tile_pool(name="zt", bufs=2) as ztp,
            tc.tile_pool(name="vv", bufs=2) as vp,
            tc.tile_pool(name="en", bufs=2) as enp,
            tc.tile_pool(name="pu", bufs=1, space="PSUM") as pup,
            tc.tile_pool(name="pe", bufs=1, space="PSUM") as pep,
        ):
            acst = {}
            for n, ap in Ain.items():
                t = constp.tile([128, 128], ddt, tag=n, name=f"c{n}")
                nc.sync.dma_start(out=t, in_=ap)
                acst[n] = t
            wcst = {}
            for n, ap in Win.items():
                t = constp.tile([128, D], ddt, tag=n, name=f"c{n}")
                nc.sync.dma_start(out=t, in_=ap)
                wcst[n] = t
            b1v = constp.tile([128, 1], f32)
            nc.sync.dma_start(out=b1v, in_=B1V)
            b2v = constp.tile([128, 1], f32)
            nc.sync.dma_start(out=b2v, in_=B2V)

            for i in range(nmt):
                zts = {}
                for n in zs:
                    znat = znp.tile([128, F], ddt, tag=f"zn{n}")
                    nc.sync.dma_start(out=znat, in_=Zv[n][i])
                    zt = ztp.tile([128, F], ddt, tag=f"zt{n}")
                    nc.vector.transpose(zt, znat)
                    zts[n] = zt
                # mm1 accumulation passes: AH@ZH (+ AL@ZH if A split, + AH@ZL)
                passes1 = [("A4H", "ZH")]
                if split_a:
                    passes1 += [("A4L", "ZH")]
                if split_z:
                    passes1 += [("A4H", "ZL")]
                passes3 = ["W2H"] + (["W2L"] if split_w else [])

                enat = enp.tile([128, F], f32)
                for t in range(nblk):
                    sl = slice(512 * t, 512 * (t + 1))
                    # strips 0-2 double-buffered (PE runs a block ahead of
                    # the relu), strip 3 single (DVE-relu) -> 7 banks + eps 1
                    ugs = [pup.tile([128, 512], f32, tag=f"ug{a}",
                                    name=f"ug{a}", bufs=(1 if a == 3 else 2))
                           for a in range(4)]
                    for pi, (wn, zn_) in enumerate(passes1):
                        w = acst[wn]
                        zt_ = zts[zn_]
                        for a in range(4):
                            nc.tensor.matmul(
                                ugs[a],
                                lhsT=w[32 * a:32 * (a + 1), :],
                                rhs=zt_[32 * a:32 * (a + 1), sl],
                                start=(pi == 0),
                                stop=(pi == len(passes1) - 1),
                                tile_position=(32 * a, 0),
                            )
                    vgs = []
                    for a in range(4):
                        vg = vp.tile([128, 512], ddt, tag=f"vg{a}")
                        if a >= 4 - DVE_RELU:
                            nc.vector.tensor_scalar(
                                vg, ugs[a], b1v, 0.0,
                                mybir.AluOpType.add, mybir.AluOpType.max)
                        else:
                            nc.scalar.activation(
                                vg, ugs[a], mybir.ActivationFunctionType.Relu,
                                bias=b1v, scale=1.0)
                        vgs.append(vg)
                    eps = pep.tile([128, 512], f32)
                    for a in range(4):
                        for pi, wn in enumerate(passes3):
                            nc.tensor.matmul(
                                eps[32 * a:32 * (a + 1), :],
                                lhsT=wcst[wn],
                                rhs=vgs[a],
                                start=(pi == 0),
                                stop=(pi == len(passes3) - 1),
                                tile_position=(0, 32 * a),
                            )
                    if b2_zero and PSUM_T:
                        nc.vector.transpose(enat[:, sl], eps)
                    else:
                        ecp = vp.tile([128, 512], f32, tag="ecp")
                        nc.scalar.activation(
                            ecp, eps, mybir.ActivationFunctionType.Identity,
                            bias=b2v, scale=1.0)
                        nc.vector.transpose(enat[:, sl], ecp)
                nc.sync.dma_start(out=Ev[i], in_=enat)

    nc.compile()
    _module_cache[key] = nc
    return nc


def _fold_params(matrix, W1, b1, W2, b2):
    """Host-side fold of the tiny params into A4/W2S/B1V/B2V (a few KB)."""
    matrix = np.asarray(matrix, np.float32)
    W1 = np.asarray(W1, np.float32)
    b1 = np.asarray(b1, np.float32)
    W2 = np.asarray(W2, np.float32)
    b2 = np.asarray(b2, np.float32)

    alpha_est = matrix * (matrix > np.float32(0.1)).astype(np.float32)
    mask = (alpha_est > np.float32(0.1)).astype(np.float32)  # (D, D)
    cnt = mask.sum(axis=1)  # (D,)
    scale = np.where(cnt > 0, np.float32(1.0) / np.maximum(cnt, 1.0),
                     np.float32(0.0)).astype(np.float32)
    M2 = (mask.T * scale[None, :]).astype(np.float32)  # M2[j,d]

    A = np.zeros((D, D * H), np.float32)
    for h in range(H):
        Ah = M2 * W1[None, :, 1, h]  # (j, d): M2[j,d] * W1[d,1,h]
        Ah[np.arange(D), np.arange(D)] += W1[:, 0, h]
        A[:, D * h:D * (h + 1)] = Ah
    A4 = np.ascontiguousarray(np.tile(A, (4, 1)))  # (128, 128)

    W2S = np.zeros((D * H, D), np.float32)
    W2S[np.arange(D * H), np.tile(np.arange(D), H)] = W2.T.reshape(-1)
    B1V = np.ascontiguousarray(b1.T.reshape(D * H, 1))
    B2V = np.ascontiguousarray(np.tile(b2, H).reshape(D * H, 1))
    b2_zero = not np.any(b2)
    return A4, W2S, B1V, B2V, b2_zero


def _split_bf16(x):
    hi = x.astype(BF16)
    lo = (x - hi.astype(np.float32)).astype(BF16)
    return np.ascontiguousarray(hi), np.ascontiguousarray(lo)


def _bf16_inputs(Z, A4, W2S):
    """Per-PREC bf16 device inputs: 0 none, 1 A-split, 2 Z+A, 3 Z+A+W2."""
    cst = {}
    if PREC >= 2:
        zh, zl = _split_bf16(Z)
        zdata = {"ZH": zh, "ZL": zl}
    else:
        zdata = {"ZH": np.ascontiguousarray(Z.astype(BF16))}
    if PREC >= 1:
        cst["A4H"], cst["A4L"] = _split_bf16(A4)
    else:
        cst["A4H"] = np.ascontiguousarray(A4.astype(BF16))
    if PREC >= 3:
        cst["W2H"], cst["W2L"] = _split_bf16(W2S)
    else:
        cst["W2H"] = np.ascontiguousarray(W2S.astype(BF16))
    return zdata, cst


def _run(Z, matrix, W1, b1, W2, b2, trace=False):
    Z = np.ascontiguousarray(np.asarray(Z, np.float32))
    assert Z.shape == (B_TOTAL, D), Z.shape
    A4, W2S, B1V, B2V, b2_zero = _fold_params(matrix, W1, b1, W2, b2)
    nc = _build_module(R, b2_zero)

    cst = {"B1V": B1V, "B2V": B2V}
    if MM_MODE == "bf16":
        zdata, cst2 = _bf16_inputs(Z, A4, W2S)
        cst.update(cst2)
    else:
        zdata = {"ZH": Z}
        cst["A4H"] = A4
        cst["W2H"] = W2S

    in_maps = [
        {**cst, **{n: z[c * R:(c + 1) * R] for n, z in zdata.items()}}
        for c in range(NCORES)
    ]
    res = bass_utils.run_bass_kernel_spmd(
        nc, in_maps, core_ids=list(range(NCORES)), trace=trace)
    out = np.concatenate([r["E"] for r in res.results], axis=0)
    return out, res


def kernel(Z, matrix, W1, b1, W2, b2):
    out, _ = _run(Z, matrix, W1, b1, W2, b2, trace=False)
    return out
